# revision 21
# baseline (speedup 1.0000x reference)
"""DeepSeekMoE kernel for 8x Trainium2 NeuronCores (Bass/Tile).

Expert-parallel sharding: core c owns routed experts {2c, 2c+1}. The host
dispatches (gathers) each expert's routed tokens to its owning core
(capacity-padded, pre-transposed so no on-device transposes are needed),
cores run the expert FFNs + a data-parallel slice of the shared experts +
per-token routing weights + aux-loss partial stats on device, and the host
un-shards: slice concat + scatter-add of weighted expert outputs.

All matmuls run in fp32r (full PE rate at moving dim >= 256, ~1e-4 rel err).
The gate matrix is column-permuted per core so the same SPMD program finds
its own two experts at columns 0 and 1.

Hardcoded for the fixed problem size:
  B,T,D = 2,2048,2048  H=1408  E=16 routed (top-2)  S=2 shared  8 cores.
"""

import numpy as np

# ---- problem dims (hardcoded) ----
B, T, D, H, E, S, TOPK = 2, 2048, 2048, 1408, 16, 2, 2
NCORES = 8
TT = B * T            # 4096 tokens
SLICE = 1024          # tokens per core for its (single) shared expert:
                      # core c runs shared expert c%2 over block c//2
EL = E // NCORES      # 2 routed experts per core
C = 640               # per-expert token capacity (max observed count 542)
P = 128
KD = D // P           # 16
KH = H // P           # 11
CT = C // P           # 5
NDC = D // 512        # 4 output chunks
C_CHUNKS = [(0, 384), (384, 256)]   # fp32r wants moving dim >= 256
S_CHUNKS = [(0, 512), (512, 512)]
ST = SLICE // P       # 8

_CACHE = {}


def _build_nc():
    import concourse.mybir as mybir
    import concourse.tile as tile
    from concourse import bacc

    F32 = mybir.dt.float32
    F32R = mybir.dt.float32r
    AF = mybir.ActivationFunctionType
    ALU = mybir.AluOpType
    AX = mybir.AxisListType

    nc = bacc.Bacc(None, target_bir_lowering=False)

    # inputs (per-core data, same shapes on every core; pre-tiled on host)
    xgT = nc.dram_tensor("xgT", [EL, P, KD, C], F32R, kind="ExternalInput")
    wmask = nc.dram_tensor("wmask", [EL, P, CT], F32, kind="ExternalInput")
    xsT = nc.dram_tensor("xsT", [P, KD, SLICE], F32R, kind="ExternalInput")
    gw = nc.dram_tensor("gw", [P, KD, E], F32R, kind="ExternalInput")
    ew1 = nc.dram_tensor("ew1", [EL, KH, P, KD, P], F32R, kind="ExternalInput")
    ew2 = nc.dram_tensor("ew2", [EL, NDC, P, KH, 512], F32R, kind="ExternalInput")
    sw1 = nc.dram_tensor("sw1", [KH, P, KD, P], F32R, kind="ExternalInput")
    sw2 = nc.dram_tensor("sw2", [NDC, P, KH, 512], F32R, kind="ExternalInput")
    # outputs
    eout = nc.dram_tensor("eout", [EL, C, D], F32, kind="ExternalOutput")
    sout = nc.dram_tensor("sout", [SLICE, D], F32, kind="ExternalOutput")
    stats = nc.dram_tensor("stats", [1, 2 * E], F32, kind="ExternalOutput")

    with tile.TileContext(nc) as tc:
        with (
            tc.tile_pool(name="const", bufs=1) as const,
            tc.tile_pool(name="wload", bufs=3) as wload,
            tc.tile_pool(name="evict", bufs=4) as evict,
            tc.tile_pool(name="small", bufs=4) as small,
            tc.tile_pool(name="pbig", bufs=4, space="PSUM") as pbig,
            tc.tile_pool(name="psmall", bufs=2, space="PSUM") as psmall,
        ):
            gw_sb = const.tile([P, KD, E], F32R)
            nc.sync.dma_start(gw_sb[:], gw.ap())
            ones_sb = const.tile([P, 1], F32)
            nc.any.memset(ones_sb[:], 1.0)

            # dummy matmuls at kernel start: keep the PE busy through the DMA
            # ramp so the HAM clock-gate reaches 8/8 before real work arrives
            warm_sb = const.tile([P, 512], F32)
            nc.vector.memset(warm_sb[:], 0.0)
            pwarm = psmall.tile([P, 512], F32, tag="pwarm", bufs=1)
            for _ in range(16):
                nc.tensor.matmul(pwarm[:], warm_sb[:, :P], warm_sb[:], start=True, stop=True)

            def routing_weights(e, xg_sb):
                """Per-token weight for local expert e (gate col e) over its
                gathered tokens; returns [P, CT] tile."""
                wm_sb = small.tile([P, CT], F32, tag="wm", bufs=2)
                nc.sync.dma_start(wm_sb[:], wmask.ap()[e])
                wgt_sb = small.tile([P, CT], F32, tag="wgt", bufs=2)
                for cm in range(CT):
                    psr = psmall.tile([P, E], F32, tag="psr")
                    for k in range(KD):
                        nc.tensor.matmul(
                            psr[:], xg_sb[:, k, cm * P : (cm + 1) * P], gw_sb[:, k],
                            start=(k == 0), stop=(k == KD - 1),
                        )
                    ext = small.tile([P, E], F32, tag="ext")
                    nc.scalar.activation(ext[:], psr[:], AF.Exp)
                    m1 = small.tile([P, 1], F32, tag="m1")
                    nc.vector.reduce_max(m1[:], ext[:], axis=AX.X)
                    mk = small.tile([P, E], F32, tag="mk")
                    nc.vector.tensor_scalar(mk[:], ext[:], m1[:], None, op0=ALU.is_equal)
                    nc.vector.tensor_tensor(mk[:], ext[:], mk[:], ALU.mult)
                    nc.vector.tensor_tensor(mk[:], ext[:], mk[:], ALU.subtract)
                    m2 = small.tile([P, 1], F32, tag="m2")
                    nc.vector.reduce_max(m2[:], mk[:], axis=AX.X)
                    nc.vector.tensor_tensor(m1[:], m1[:], m2[:], ALU.add)
                    rc = small.tile([P, 1], F32, tag="rc")
                    nc.vector.reciprocal(rc[:], m1[:])
                    nc.vector.tensor_tensor(rc[:], ext[:, e : e + 1], rc[:], ALU.mult)
                    nc.vector.tensor_tensor(
                        wgt_sb[:, cm : cm + 1], rc[:], wm_sb[:, cm : cm + 1], ALU.mult
                    )
                return wgt_sb

            def expert_mm1(xg_sb, se_sb, w1_ap):
                for ho in range(KH):
                    w1t = wload.tile([P, KD, P], F32R, tag="w1strip")
                    nc.sync.dma_start(w1t[:], w1_ap[ho])
                    for c0, cw in C_CHUNKS:
                        ps1 = pbig.tile([P, 512], F32, tag="ps")
                        for k in range(KD):
                            nc.tensor.matmul(
                                ps1[:, :cw], w1t[:, k], xg_sb[:, k, c0 : c0 + cw],
                                start=(k == 0), stop=(k == KD - 1),
                            )
                        nc.scalar.activation(
                            se_sb[:, ho, c0 : c0 + cw], ps1[:, :cw], AF.Silu
                        )

            def expert_mm2(e, se_sb, wgt_sb):
                for dch in range(NDC):
                    w2t = wload.tile([P, KH, 512], F32R, tag="w2s", bufs=2,
                                     name=f"ew2t_{e}_{dch}")
                    nc.sync.dma_start(w2t[:], ew2.ap()[e, dch])
                    for cm in range(CT):
                        ps2 = pbig.tile([P, 512], F32, tag="ps")
                        for ho in range(KH):
                            nc.tensor.matmul(
                                ps2[:],
                                se_sb[:, ho, cm * P : (cm + 1) * P],
                                w2t[:, ho],
                                start=(ho == 0), stop=(ho == KH - 1),
                            )
                        eo_sb = evict.tile([P, 512], F32, tag="ev")
                        nc.vector.tensor_scalar(
                            eo_sb[:], ps2[:], wgt_sb[:, cm : cm + 1], None, op0=ALU.mult
                        )
                        nc.sync.dma_start(
                            eout.ap()[
                                e, cm * P : (cm + 1) * P, dch * 512 : (dch + 1) * 512
                            ],
                            eo_sb[:],
                        )

            # ============ emission: shared+aux first, expert loads overlapped ============
            ss_ctx = tc.tile_pool(name="ssp", bufs=1)
            ssp = ss_ctx.__enter__()
            ss_sb = ssp.tile([P, KH, SLICE], F32R)

            xs_ctx = tc.tile_pool(name="xsp", bufs=1)
            xsp = xs_ctx.__enter__()
            xs_sb = xsp.tile([P, KD, SLICE], F32R)
            for k in range(KD):
                nc.sync.dma_start(xs_sb[:, k], xsT.ap()[:, k])

            # ---- aux-loss partial stats ----
            # Over the FIRST 512 tokens of this core's block only: the host
            # orders each odd core's block so the two cores of a block cover
            # disjoint halves (every token counted exactly once fleet-wide).
            lacc = small.tile([P, E], F32, bufs=1)
            pacc = small.tile([P, E], F32, bufs=1)
            nc.vector.memset(lacc[:], 0.0)
            nc.vector.memset(pacc[:], 0.0)
            for cm in range(512 // P):
                psr = psmall.tile([P, E], F32, tag="psr")
                for k in range(KD):
                    nc.tensor.matmul(
                        psr[:], xs_sb[:, k, cm * P : (cm + 1) * P], gw_sb[:, k],
                        start=(k == 0), stop=(k == KD - 1),
                    )
                lt = small.tile([P, E], F32, tag="lt")
                nc.vector.tensor_copy(lt[:], psr[:])
                nc.vector.tensor_tensor(lacc[:], lacc[:], lt[:], ALU.add)
                ext = small.tile([P, E], F32, tag="ext")
                nc.scalar.activation(ext[:], psr[:], AF.Exp)
                rs = small.tile([P, 1], F32, tag="rs")
                nc.vector.reduce_sum(rs[:], ext[:], axis=AX.X)
                rc = small.tile([P, 1], F32, tag="rc")
                nc.vector.reciprocal(rc[:], rs[:])
                pt = small.tile([P, E], F32, tag="pt")
                nc.vector.tensor_scalar(pt[:], ext[:], rc[:], None, op0=ALU.mult)
                nc.vector.tensor_tensor(pacc[:], pacc[:], pt[:], ALU.add)
            st_sb = small.tile([1, 2 * E], F32, bufs=1)
            psa = psmall.tile([1, E], F32, tag="psr")
            nc.tensor.matmul(psa[:], ones_sb[:], pacc[:], start=True, stop=True)
            nc.vector.tensor_copy(st_sb[:, 0:E], psa[:])
            psb = psmall.tile([1, E], F32, tag="psr")
            nc.tensor.matmul(psb[:], ones_sb[:], lacc[:], start=True, stop=True)
            nc.vector.tensor_copy(st_sb[:, E : 2 * E], psb[:])
            nc.sync.dma_start(stats.ap(), st_sb[:])

            # ---- shared matmul1 + silu (one shared expert, 1024-token block) ----
            for ho in range(KH):
                w1t = wload.tile([P, KD, P], F32R, tag="w1strip")
                nc.sync.dma_start(w1t[:], sw1.ap()[ho])
                for c0, cw in S_CHUNKS:
                    ps1 = pbig.tile([P, 512], F32, tag="ps")
                    for k in range(KD):
                        nc.tensor.matmul(
                            ps1[:, :cw], w1t[:, k], xs_sb[:, k, c0 : c0 + cw],
                            start=(k == 0), stop=(k == KD - 1),
                        )
                    nc.scalar.activation(ss_sb[:, ho, c0 : c0 + cw], ps1[:, :cw], AF.Silu)
            xs_ctx.__exit__(None, None, None)  # free xs range -> xg reuses it

            # ---- expert 0 gather + routing, overlapped with shared matmul2 ----
            xg_ctx = tc.tile_pool(name="xgp", bufs=1, side="right")
            xgp = xg_ctx.__enter__()
            xg0 = xgp.tile([P, KD, C], F32R, tag="xg", name="xg0")
            for k in range(KD):
                nc.sync.dma_start(xg0[:, k], xgT.ap()[0, :, k])
            wgt0 = routing_weights(0, xg0)

            # ---- shared matmul2 ----
            for dch in range(NDC):
                w2t = wload.tile([P, KH, 512], F32R, tag="w2s", bufs=2)
                nc.sync.dma_start(w2t[:], sw2.ap()[dch])
                for cm in range(ST):
                    ps2 = pbig.tile([P, 512], F32, tag="ps")
                    for ho in range(KH):
                        nc.tensor.matmul(
                            ps2[:],
                            ss_sb[:, ho, cm * P : (cm + 1) * P],
                            w2t[:, ho],
                            start=(ho == 0), stop=(ho == KH - 1),
                        )
                    so_sb = evict.tile([P, 512], F32, tag="ev")
                    nc.scalar.activation(so_sb[:], ps2[:], AF.Copy)
                    nc.sync.dma_start(
                        sout.ap()[cm * P : (cm + 1) * P, dch * 512 : (dch + 1) * 512],
                        so_sb[:],
                    )
            ss_ctx.__exit__(None, None, None)  # free ss range -> se reuses it

            # ---- routed experts ----
            se_ctx = tc.tile_pool(name="sep", bufs=1)
            sep = se_ctx.__enter__()
            se0 = sep.tile([P, KH, C], F32R, tag="se", name="se0")
            expert_mm1(xg0, se0, ew1.ap()[0])

            # expert 1 gather + routing emitted before e0 mm2 so its DMA queues early
            xg1 = xgp.tile([P, KD, C], F32R, tag="xg", name="xg1")
            for k in range(KD):
                nc.sync.dma_start(xg1[:, k], xgT.ap()[1, :, k])
            wgt1 = routing_weights(1, xg1)

            expert_mm2(0, se0, wgt0)

            se1 = sep.tile([P, KH, C], F32R, tag="se", name="se1")
            expert_mm1(xg1, se1, ew1.ap()[1])
            expert_mm2(1, se1, wgt1)

            se_ctx.__exit__(None, None, None)
            xg_ctx.__exit__(None, None, None)
    nc.compile()
    return nc


def _get_nc():
    if "nc" not in _CACHE:
        _CACHE["nc"] = _build_nc()
    return _CACHE["nc"]


def _host_route(xf, gate_w):
    """Duplicate of the router, for dispatch indices only."""
    logits = xf @ gate_w                                   # [TT, E]
    order = np.argsort(-logits, axis=1, kind="stable")     # ties: lower index first
    top2 = order[:, :TOPK]
    idx_lists = []
    for e in range(E):
        members = np.nonzero((top2 == e).any(axis=1))[0]
        idx_lists.append(members.astype(np.int64))
    return idx_lists


def _tile_w1(w):   # [D, H] -> [KH, P, KD, P]
    return np.ascontiguousarray(w.reshape(KD, P, KH, P).transpose(2, 1, 0, 3))


def _tile_w2(w):   # [H, D] -> [NDC, P, KH, 512]
    return np.ascontiguousarray(w.reshape(KH, P, NDC, 512).transpose(2, 1, 0, 3))


def _tile_kx(a):   # [D, N] -> [P, KD, N]
    return np.ascontiguousarray(a.reshape(KD, P, -1).transpose(1, 0, 2))


def kernel(x, gate_w, shared_w1, shared_w2, expert_w1, expert_w2):
    from concourse.bass_utils import run_bass_kernel_spmd

    x = np.asarray(x, dtype=np.float32)
    gate_w = np.asarray(gate_w, dtype=np.float32)
    shared_w1 = np.asarray(shared_w1, dtype=np.float32)
    shared_w2 = np.asarray(shared_w2, dtype=np.float32)
    expert_w1 = np.asarray(expert_w1, dtype=np.float32)
    expert_w2 = np.asarray(expert_w2, dtype=np.float32)

    xf = np.ascontiguousarray(x.reshape(TT, D))
    idx_lists = _host_route(xf, gate_w)
    for e, il in enumerate(idx_lists):
        if len(il) > C:
            raise RuntimeError(f"expert {e} count {len(il)} exceeds capacity {C}")

    sw1_dev = [_tile_w1(shared_w1[s]) for s in range(S)]
    sw2_dev = [_tile_w2(shared_w2[s]) for s in range(S)]
    # per-block token batch; odd cores get the half-swapped order so their
    # aux pass (first 512 columns) covers the block's second half
    xs_even = [
        _tile_kx(xf[b * SLICE : (b + 1) * SLICE].T) for b in range(TT // SLICE)
    ]
    xs_odd = [
        _tile_kx(
            np.concatenate(
                [
                    xf[b * SLICE + SLICE // 2 : (b + 1) * SLICE],
                    xf[b * SLICE : b * SLICE + SLICE // 2],
                ]
            ).T
        )
        for b in range(TT // SLICE)
    ]

    in_maps = []
    perms = []
    for c in range(NCORES):
        el = [2 * c, 2 * c + 1]
        perm = el + [j for j in range(E) if j not in el]
        perms.append(perm)
        xgT = np.zeros((EL, P, KD, C), dtype=np.float32)
        wmask = np.zeros((EL, P, CT), dtype=np.float32)
        for i, e in enumerate(el):
            il = idx_lists[e]
            n = len(il)
            g = np.zeros((D, C), dtype=np.float32)
            g[:, :n] = xf[il].T
            xgT[i] = _tile_kx(g)
            wm = np.zeros(C, dtype=np.float32)
            wm[:n] = 1.0
            wmask[i] = wm.reshape(CT, P).T
        in_maps.append(
            {
                "xgT": xgT,
                "wmask": wmask,
                "xsT": (xs_even if c % 2 == 0 else xs_odd)[c // 2],
                "gw": _tile_kx(gate_w[:, perm]),
                "ew1": np.stack([_tile_w1(expert_w1[e]) for e in el]),
                "ew2": np.stack([_tile_w2(expert_w2[e]) for e in el]),
                "sw1": sw1_dev[c % 2],       # shared expert c%2
                "sw2": sw2_dev[c % 2],
            }
        )

    nc = _get_nc()
    res = run_bass_kernel_spmd(nc, in_maps, core_ids=list(range(NCORES)))
    results = res.results

    # un-shard: each token block's shared output = sum of its two cores'
    # (one shared expert each), then scatter-add weighted expert outputs
    total = np.empty((TT, D), dtype=np.float32)
    h = SLICE // 2
    for b in range(TT // SLICE):
        so_odd = results[2 * b + 1]["sout"]
        total[b * SLICE : (b + 1) * SLICE] = results[2 * b]["sout"] + np.concatenate(
            [so_odd[h:], so_odd[:h]]
        )
    for c in range(NCORES):
        for i in range(EL):
            e = 2 * c + i
            il = idx_lists[e]
            n = len(il)
            total[il] += results[c]["eout"][i, :n]

    # aux loss from per-core block stats (each token counted on 2 cores;
    # device cols are permuted per core)
    psum = np.zeros(E, dtype=np.float64)
    lsum = np.zeros(E, dtype=np.float64)
    for c in range(NCORES):
        st = results[c]["stats"][0]
        psum[perms[c]] += st[:E]
        lsum[perms[c]] += st[E:]
    aux = np.float32((psum / TT * (lsum / TT)).sum() * E)

    return total.reshape(B, T, D), aux


# revision 24
# speedup vs baseline: 1.0063x; 1.0063x over previous
"""DeepSeekMoE kernel for 8x Trainium2 NeuronCores (Bass/Tile).

Expert-parallel sharding: core c owns routed experts {2c, 2c+1}. The host
dispatches (gathers) each expert's routed tokens to its owning core
(capacity-padded, pre-transposed so no on-device transposes are needed),
cores run the expert FFNs + a data-parallel slice of the shared experts +
per-token routing weights + aux-loss partial stats on device, and the host
un-shards: slice concat + scatter-add of weighted expert outputs.

All matmuls run in fp32r (full PE rate at moving dim >= 256, ~1e-4 rel err).
The gate matrix is column-permuted per core so the same SPMD program finds
its own two experts at columns 0 and 1.

Hardcoded for the fixed problem size:
  B,T,D = 2,2048,2048  H=1408  E=16 routed (top-2)  S=2 shared  8 cores.
"""

import numpy as np

# ---- problem dims (hardcoded) ----
B, T, D, H, E, S, TOPK = 2, 2048, 2048, 1408, 16, 2, 2
NCORES = 8
TT = B * T            # 4096 tokens
SLICE = 1024          # tokens per core for its (single) shared expert:
                      # core c runs shared expert c%2 over block c//2
EL = E // NCORES      # 2 routed experts per core
C = 640               # per-expert token capacity (max observed count 542)
P = 128
KD = D // P           # 16
KH = H // P           # 11
CT = C // P           # 5
NDC = D // 512        # 4 output chunks
C_CHUNKS = [(0, 384), (384, 256)]   # fp32r wants moving dim >= 256
S_CHUNKS = [(0, 512), (512, 512)]
ST = SLICE // P       # 8

_CACHE = {}


def _build_nc():
    import concourse.mybir as mybir
    import concourse.tile as tile
    from concourse import bacc

    F32 = mybir.dt.float32
    F32R = mybir.dt.float32r
    AF = mybir.ActivationFunctionType
    ALU = mybir.AluOpType
    AX = mybir.AxisListType

    nc = bacc.Bacc(None, target_bir_lowering=False)

    # inputs (per-core data, same shapes on every core; pre-tiled on host)
    xgT = nc.dram_tensor("xgT", [EL, P, KD, C], F32R, kind="ExternalInput")
    wmask = nc.dram_tensor("wmask", [EL, P, CT], F32, kind="ExternalInput")
    xsT = nc.dram_tensor("xsT", [P, KD, SLICE], F32R, kind="ExternalInput")
    gw = nc.dram_tensor("gw", [P, KD, E], F32R, kind="ExternalInput")
    ew1 = nc.dram_tensor("ew1", [EL, KH, P, KD, P], F32R, kind="ExternalInput")
    ew2 = nc.dram_tensor("ew2", [EL, NDC, P, KH, 512], F32R, kind="ExternalInput")
    sw1 = nc.dram_tensor("sw1", [KH, P, KD, P], F32R, kind="ExternalInput")
    sw2 = nc.dram_tensor("sw2", [NDC, P, KH, 512], F32R, kind="ExternalInput")
    # outputs
    eout = nc.dram_tensor("eout", [EL, C, D], F32, kind="ExternalOutput")
    sout = nc.dram_tensor("sout", [SLICE, D], F32, kind="ExternalOutput")
    stats = nc.dram_tensor("stats", [1, 2 * E], F32, kind="ExternalOutput")

    with tile.TileContext(nc) as tc:
        with (
            tc.tile_pool(name="const", bufs=1) as const,
            tc.tile_pool(name="wload", bufs=3) as wload,
            tc.tile_pool(name="evict", bufs=4) as evict,
            tc.tile_pool(name="small", bufs=4) as small,
            tc.tile_pool(name="pbig", bufs=4, space="PSUM") as pbig,
            tc.tile_pool(name="psmall", bufs=2, space="PSUM") as psmall,
        ):
            gw_sb = const.tile([P, KD, E], F32R)
            nc.sync.dma_start(gw_sb[:], gw.ap())
            ones_sb = const.tile([P, 1], F32)
            nc.any.memset(ones_sb[:], 1.0)

            # dummy matmuls at kernel start: keep the PE busy through the DMA
            # ramp so the HAM clock-gate reaches 8/8 before real work arrives
            warm_sb = const.tile([P, 512], F32)
            nc.vector.memset(warm_sb[:], 0.0)
            pwarm = psmall.tile([P, 512], F32, tag="pwarm", bufs=1)
            for _ in range(24):
                nc.tensor.matmul(pwarm[:], warm_sb[:, :P], warm_sb[:], start=True, stop=True)

            def routing_weights(e, xg_sb):
                """Per-token weight for local expert e (gate col e) over its
                gathered tokens; returns [P, CT] tile."""
                wm_sb = small.tile([P, CT], F32, tag="wm", bufs=2)
                nc.sync.dma_start(wm_sb[:], wmask.ap()[e])
                wgt_sb = small.tile([P, CT], F32, tag="wgt", bufs=2)
                for cm in range(CT):
                    psr = psmall.tile([P, E], F32, tag="psr")
                    for k in range(KD):
                        nc.tensor.matmul(
                            psr[:], xg_sb[:, k, cm * P : (cm + 1) * P], gw_sb[:, k],
                            start=(k == 0), stop=(k == KD - 1),
                        )
                    ext = small.tile([P, E], F32, tag="ext")
                    nc.scalar.activation(ext[:], psr[:], AF.Exp)
                    m1 = small.tile([P, 1], F32, tag="m1")
                    nc.vector.reduce_max(m1[:], ext[:], axis=AX.X)
                    mk = small.tile([P, E], F32, tag="mk")
                    nc.vector.tensor_scalar(mk[:], ext[:], m1[:], None, op0=ALU.is_equal)
                    nc.vector.tensor_tensor(mk[:], ext[:], mk[:], ALU.mult)
                    nc.vector.tensor_tensor(mk[:], ext[:], mk[:], ALU.subtract)
                    m2 = small.tile([P, 1], F32, tag="m2")
                    nc.vector.reduce_max(m2[:], mk[:], axis=AX.X)
                    nc.vector.tensor_tensor(m1[:], m1[:], m2[:], ALU.add)
                    rc = small.tile([P, 1], F32, tag="rc")
                    nc.vector.reciprocal(rc[:], m1[:])
                    nc.vector.tensor_tensor(rc[:], ext[:, e : e + 1], rc[:], ALU.mult)
                    nc.vector.tensor_tensor(
                        wgt_sb[:, cm : cm + 1], rc[:], wm_sb[:, cm : cm + 1], ALU.mult
                    )
                return wgt_sb

            def expert_mm1(xg_sb, se_sb, w1_ap):
                for ho in range(KH):
                    w1t = wload.tile([P, KD, P], F32R, tag="w1strip")
                    nc.sync.dma_start(w1t[:], w1_ap[ho])
                    for c0, cw in C_CHUNKS:
                        ps1 = pbig.tile([P, 512], F32, tag="ps")
                        for k in range(KD):
                            nc.tensor.matmul(
                                ps1[:, :cw], w1t[:, k], xg_sb[:, k, c0 : c0 + cw],
                                start=(k == 0), stop=(k == KD - 1),
                            )
                        nc.scalar.activation(
                            se_sb[:, ho, c0 : c0 + cw], ps1[:, :cw], AF.Silu
                        )

            def expert_mm2(e, se_sb, wgt_sb):
                for dch in range(NDC):
                    w2t = wload.tile([P, KH, 512], F32R, tag="w2s", bufs=2,
                                     name=f"ew2t_{e}_{dch}")
                    nc.sync.dma_start(w2t[:], ew2.ap()[e, dch])
                    for cm in range(CT):
                        ps2 = pbig.tile([P, 512], F32, tag="ps")
                        for ho in range(KH):
                            nc.tensor.matmul(
                                ps2[:],
                                se_sb[:, ho, cm * P : (cm + 1) * P],
                                w2t[:, ho],
                                start=(ho == 0), stop=(ho == KH - 1),
                            )
                        eo_sb = evict.tile([P, 512], F32, tag="ev")
                        nc.vector.tensor_scalar(
                            eo_sb[:], ps2[:], wgt_sb[:, cm : cm + 1], None, op0=ALU.mult
                        )
                        nc.sync.dma_start(
                            eout.ap()[
                                e, cm * P : (cm + 1) * P, dch * 512 : (dch + 1) * 512
                            ],
                            eo_sb[:],
                        )

            # ============ emission: shared+aux first, expert loads overlapped ============
            ss_ctx = tc.tile_pool(name="ssp", bufs=1)
            ssp = ss_ctx.__enter__()
            ss_sb = ssp.tile([P, KH, SLICE], F32R)

            xs_ctx = tc.tile_pool(name="xsp", bufs=1)
            xsp = xs_ctx.__enter__()
            xs_sb = xsp.tile([P, KD, SLICE], F32R)
            for k in range(KD):
                nc.sync.dma_start(xs_sb[:, k], xsT.ap()[:, k])

            # ---- aux-loss partial stats ----
            # Over the FIRST 512 tokens of this core's block only: the host
            # orders each odd core's block so the two cores of a block cover
            # disjoint halves (every token counted exactly once fleet-wide).
            lacc = small.tile([P, E], F32, bufs=1)
            pacc = small.tile([P, E], F32, bufs=1)
            nc.vector.memset(lacc[:], 0.0)
            nc.vector.memset(pacc[:], 0.0)
            for cm in range(512 // P):
                psr = psmall.tile([P, E], F32, tag="psr")
                for k in range(KD):
                    nc.tensor.matmul(
                        psr[:], xs_sb[:, k, cm * P : (cm + 1) * P], gw_sb[:, k],
                        start=(k == 0), stop=(k == KD - 1),
                    )
                lt = small.tile([P, E], F32, tag="lt")
                nc.vector.tensor_copy(lt[:], psr[:])
                nc.vector.tensor_tensor(lacc[:], lacc[:], lt[:], ALU.add)
                ext = small.tile([P, E], F32, tag="ext")
                nc.scalar.activation(ext[:], psr[:], AF.Exp)
                rs = small.tile([P, 1], F32, tag="rs")
                nc.vector.reduce_sum(rs[:], ext[:], axis=AX.X)
                rc = small.tile([P, 1], F32, tag="rc")
                nc.vector.reciprocal(rc[:], rs[:])
                pt = small.tile([P, E], F32, tag="pt")
                nc.vector.tensor_scalar(pt[:], ext[:], rc[:], None, op0=ALU.mult)
                nc.vector.tensor_tensor(pacc[:], pacc[:], pt[:], ALU.add)
            st_sb = small.tile([1, 2 * E], F32, bufs=1)
            psa = psmall.tile([1, E], F32, tag="psr")
            nc.tensor.matmul(psa[:], ones_sb[:], pacc[:], start=True, stop=True)
            nc.vector.tensor_copy(st_sb[:, 0:E], psa[:])
            psb = psmall.tile([1, E], F32, tag="psr")
            nc.tensor.matmul(psb[:], ones_sb[:], lacc[:], start=True, stop=True)
            nc.vector.tensor_copy(st_sb[:, E : 2 * E], psb[:])
            nc.sync.dma_start(stats.ap(), st_sb[:])

            # ---- shared matmul1 + silu (one shared expert, 1024-token block) ----
            for ho in range(KH):
                w1t = wload.tile([P, KD, P], F32R, tag="w1strip")
                nc.sync.dma_start(w1t[:], sw1.ap()[ho])
                for c0, cw in S_CHUNKS:
                    ps1 = pbig.tile([P, 512], F32, tag="ps")
                    for k in range(KD):
                        nc.tensor.matmul(
                            ps1[:, :cw], w1t[:, k], xs_sb[:, k, c0 : c0 + cw],
                            start=(k == 0), stop=(k == KD - 1),
                        )
                    nc.scalar.activation(ss_sb[:, ho, c0 : c0 + cw], ps1[:, :cw], AF.Silu)
            xs_ctx.__exit__(None, None, None)  # free xs range -> xg reuses it

            # ---- expert 0 gather + routing, overlapped with shared matmul2 ----
            xg_ctx = tc.tile_pool(name="xgp", bufs=1, side="right")
            xgp = xg_ctx.__enter__()
            xg0 = xgp.tile([P, KD, C], F32R, tag="xg", name="xg0")
            for k in range(KD):
                nc.sync.dma_start(xg0[:, k], xgT.ap()[0, :, k])
            wgt0 = routing_weights(0, xg0)

            # ---- shared matmul2 ----
            for dch in range(NDC):
                w2t = wload.tile([P, KH, 512], F32R, tag="w2s", bufs=2)
                nc.sync.dma_start(w2t[:], sw2.ap()[dch])
                for cm in range(ST):
                    ps2 = pbig.tile([P, 512], F32, tag="ps")
                    for ho in range(KH):
                        nc.tensor.matmul(
                            ps2[:],
                            ss_sb[:, ho, cm * P : (cm + 1) * P],
                            w2t[:, ho],
                            start=(ho == 0), stop=(ho == KH - 1),
                        )
                    so_sb = evict.tile([P, 512], F32, tag="ev")
                    nc.scalar.activation(so_sb[:], ps2[:], AF.Copy)
                    nc.sync.dma_start(
                        sout.ap()[cm * P : (cm + 1) * P, dch * 512 : (dch + 1) * 512],
                        so_sb[:],
                    )
            ss_ctx.__exit__(None, None, None)  # free ss range -> se reuses it

            # ---- routed experts ----
            se_ctx = tc.tile_pool(name="sep", bufs=2)
            sep = se_ctx.__enter__()
            se0 = sep.tile([P, KH, C], F32R, tag="se", name="se0")
            expert_mm1(xg0, se0, ew1.ap()[0])

            # expert 1 gather + routing emitted before e0 mm2 so its DMA queues early
            xg1 = xgp.tile([P, KD, C], F32R, tag="xg", name="xg1")
            for k in range(KD):
                nc.sync.dma_start(xg1[:, k], xgT.ap()[1, :, k])
            wgt1 = routing_weights(1, xg1)

            expert_mm2(0, se0, wgt0)

            se1 = sep.tile([P, KH, C], F32R, tag="se", name="se1")
            expert_mm1(xg1, se1, ew1.ap()[1])
            expert_mm2(1, se1, wgt1)

            se_ctx.__exit__(None, None, None)
            xg_ctx.__exit__(None, None, None)
    nc.compile()
    return nc


def _get_nc():
    if "nc" not in _CACHE:
        _CACHE["nc"] = _build_nc()
    return _CACHE["nc"]


def _host_route(xf, gate_w):
    """Duplicate of the router, for dispatch indices only."""
    logits = xf @ gate_w                                   # [TT, E]
    order = np.argsort(-logits, axis=1, kind="stable")     # ties: lower index first
    top2 = order[:, :TOPK]
    idx_lists = []
    for e in range(E):
        members = np.nonzero((top2 == e).any(axis=1))[0]
        idx_lists.append(members.astype(np.int64))
    return idx_lists


def _tile_w1(w):   # [D, H] -> [KH, P, KD, P]
    return np.ascontiguousarray(w.reshape(KD, P, KH, P).transpose(2, 1, 0, 3))


def _tile_w2(w):   # [H, D] -> [NDC, P, KH, 512]
    return np.ascontiguousarray(w.reshape(KH, P, NDC, 512).transpose(2, 1, 0, 3))


def _tile_kx(a):   # [D, N] -> [P, KD, N]
    return np.ascontiguousarray(a.reshape(KD, P, -1).transpose(1, 0, 2))


def kernel(x, gate_w, shared_w1, shared_w2, expert_w1, expert_w2):
    from concourse.bass_utils import run_bass_kernel_spmd

    x = np.asarray(x, dtype=np.float32)
    gate_w = np.asarray(gate_w, dtype=np.float32)
    shared_w1 = np.asarray(shared_w1, dtype=np.float32)
    shared_w2 = np.asarray(shared_w2, dtype=np.float32)
    expert_w1 = np.asarray(expert_w1, dtype=np.float32)
    expert_w2 = np.asarray(expert_w2, dtype=np.float32)

    xf = np.ascontiguousarray(x.reshape(TT, D))
    idx_lists = _host_route(xf, gate_w)
    for e, il in enumerate(idx_lists):
        if len(il) > C:
            raise RuntimeError(f"expert {e} count {len(il)} exceeds capacity {C}")

    sw1_dev = [_tile_w1(shared_w1[s]) for s in range(S)]
    sw2_dev = [_tile_w2(shared_w2[s]) for s in range(S)]
    # per-block token batch; odd cores get the half-swapped order so their
    # aux pass (first 512 columns) covers the block's second half
    xs_even = [
        _tile_kx(xf[b * SLICE : (b + 1) * SLICE].T) for b in range(TT // SLICE)
    ]
    xs_odd = [
        _tile_kx(
            np.concatenate(
                [
                    xf[b * SLICE + SLICE // 2 : (b + 1) * SLICE],
                    xf[b * SLICE : b * SLICE + SLICE // 2],
                ]
            ).T
        )
        for b in range(TT // SLICE)
    ]

    in_maps = []
    perms = []
    for c in range(NCORES):
        el = [2 * c, 2 * c + 1]
        perm = el + [j for j in range(E) if j not in el]
        perms.append(perm)
        xgT = np.zeros((EL, P, KD, C), dtype=np.float32)
        wmask = np.zeros((EL, P, CT), dtype=np.float32)
        for i, e in enumerate(el):
            il = idx_lists[e]
            n = len(il)
            g = np.zeros((D, C), dtype=np.float32)
            g[:, :n] = xf[il].T
            xgT[i] = _tile_kx(g)
            wm = np.zeros(C, dtype=np.float32)
            wm[:n] = 1.0
            wmask[i] = wm.reshape(CT, P).T
        in_maps.append(
            {
                "xgT": xgT,
                "wmask": wmask,
                "xsT": (xs_even if c % 2 == 0 else xs_odd)[c // 2],
                "gw": _tile_kx(gate_w[:, perm]),
                "ew1": np.stack([_tile_w1(expert_w1[e]) for e in el]),
                "ew2": np.stack([_tile_w2(expert_w2[e]) for e in el]),
                "sw1": sw1_dev[c % 2],       # shared expert c%2
                "sw2": sw2_dev[c % 2],
            }
        )

    nc = _get_nc()
    last_err = None
    for attempt in range(3):
        try:
            res = run_bass_kernel_spmd(nc, in_maps, core_ids=list(range(NCORES)))
            break
        except Exception as err:  # transient device wedge -> retry
            last_err = err
            import time as _time

            _time.sleep(15.0 * (attempt + 1))
    else:
        raise last_err
    results = res.results

    # un-shard: each token block's shared output = sum of its two cores'
    # (one shared expert each), then scatter-add weighted expert outputs
    total = np.empty((TT, D), dtype=np.float32)
    h = SLICE // 2
    for b in range(TT // SLICE):
        so_odd = results[2 * b + 1]["sout"]
        total[b * SLICE : (b + 1) * SLICE] = results[2 * b]["sout"] + np.concatenate(
            [so_odd[h:], so_odd[:h]]
        )
    for c in range(NCORES):
        for i in range(EL):
            e = 2 * c + i
            il = idx_lists[e]
            n = len(il)
            total[il] += results[c]["eout"][i, :n]

    # aux loss from per-core block stats (each token counted on 2 cores;
    # device cols are permuted per core)
    psum = np.zeros(E, dtype=np.float64)
    lsum = np.zeros(E, dtype=np.float64)
    for c in range(NCORES):
        st = results[c]["stats"][0]
        psum[perms[c]] += st[:E]
        lsum[perms[c]] += st[E:]
    aux = np.float32((psum / TT * (lsum / TT)).sum() * E)

    return total.reshape(B, T, D), aux


# revision 29
# speedup vs baseline: 1.0234x; 1.0170x over previous
"""DeepSeekMoE kernel for 8x Trainium2 NeuronCores (Bass/Tile).

Expert-parallel sharding: core c owns routed experts {2c, 2c+1}. The host
dispatches (gathers) each expert's routed tokens to its owning core
(capacity-padded, pre-transposed so no on-device transposes are needed),
cores run the expert FFNs + a data-parallel slice of the shared experts +
per-token routing weights + aux-loss partial stats on device, and the host
un-shards: slice concat + scatter-add of weighted expert outputs.

All matmuls run in fp32r (full PE rate at moving dim >= 256, ~1e-4 rel err).
The gate matrix is column-permuted per core so the same SPMD program finds
its own two experts at columns 0 and 1.

Hardcoded for the fixed problem size:
  B,T,D = 2,2048,2048  H=1408  E=16 routed (top-2)  S=2 shared  8 cores.
"""

import numpy as np

# ---- problem dims (hardcoded) ----
B, T, D, H, E, S, TOPK = 2, 2048, 2048, 1408, 16, 2, 2
NCORES = 8
TT = B * T            # 4096 tokens
SLICE = 1024          # tokens per core for its (single) shared expert:
                      # core c runs shared expert c%2 over block c//2
EL = E // NCORES      # 2 routed experts per core
C = 640               # per-expert token capacity (max observed count 542)
P = 128
KD = D // P           # 16
KH = H // P           # 11
CT = C // P           # 5
NDC = D // 512        # 4 output chunks
C_CHUNKS = [(0, 384), (384, 256)]   # fp32r wants moving dim >= 256
S_CHUNKS = [(0, 512), (512, 512)]
ST = SLICE // P       # 8

_CACHE = {}


def _build_nc():
    import concourse.mybir as mybir
    import concourse.tile as tile
    from concourse import bacc

    F32 = mybir.dt.float32
    F32R = mybir.dt.float32r
    AF = mybir.ActivationFunctionType
    ALU = mybir.AluOpType
    AX = mybir.AxisListType

    nc = bacc.Bacc(None, target_bir_lowering=False)

    # inputs (per-core data, same shapes on every core; pre-tiled on host)
    xgT = nc.dram_tensor("xgT", [EL, P, KD, C], F32R, kind="ExternalInput")
    wmask = nc.dram_tensor("wmask", [EL, P, CT], F32, kind="ExternalInput")
    xsT = nc.dram_tensor("xsT", [P, KD, SLICE], F32R, kind="ExternalInput")
    gw = nc.dram_tensor("gw", [P, KD, E], F32R, kind="ExternalInput")
    ew1 = nc.dram_tensor("ew1", [EL, KH, P, KD, P], F32R, kind="ExternalInput")
    ew2 = nc.dram_tensor("ew2", [EL, NDC, P, KH, 512], F32R, kind="ExternalInput")
    sw1 = nc.dram_tensor("sw1", [KH, P, KD, P], F32R, kind="ExternalInput")
    sw2 = nc.dram_tensor("sw2", [NDC, P, KH, 512], F32R, kind="ExternalInput")
    # outputs
    eout = nc.dram_tensor("eout", [EL, C, D], F32, kind="ExternalOutput")
    sout = nc.dram_tensor("sout", [SLICE, D], F32, kind="ExternalOutput")
    stats = nc.dram_tensor("stats", [1, 2 * E], F32, kind="ExternalOutput")

    with tile.TileContext(nc) as tc:
        with (
            tc.tile_pool(name="const", bufs=1) as const,
            tc.tile_pool(name="wload", bufs=3) as wload,
            tc.tile_pool(name="evict", bufs=4) as evict,
            tc.tile_pool(name="small", bufs=4) as small,
            tc.tile_pool(name="pbig", bufs=4, space="PSUM") as pbig,
            tc.tile_pool(name="psmall", bufs=2, space="PSUM") as psmall,
        ):
            gw_sb = const.tile([P, KD, E], F32R)
            nc.sync.dma_start(gw_sb[:], gw.ap())
            ones_sb = const.tile([P, 1], F32)
            nc.any.memset(ones_sb[:], 1.0)

            # dummy matmuls at kernel start: keep the PE busy through the DMA
            # ramp so the HAM clock-gate reaches 8/8 before real work arrives
            warm_sb = const.tile([P, 512], F32)
            nc.vector.memset(warm_sb[:], 0.0)
            pwarm = psmall.tile([P, 512], F32, tag="pwarm", bufs=1)
            for _ in range(24):
                nc.tensor.matmul(pwarm[:], warm_sb[:, :P], warm_sb[:], start=True, stop=True)

            def routing_weights(e, xg_sb):
                """Per-token weight for local expert e (gate col e) over its
                gathered tokens; returns [P, CT] tile."""
                wm_sb = small.tile([P, CT], F32, tag="wm", bufs=2)
                nc.sync.dma_start(wm_sb[:], wmask.ap()[e])
                wgt_sb = small.tile([P, CT], F32, tag="wgt", bufs=2)
                for cm in range(CT):
                    psr = psmall.tile([P, E], F32, tag="psr")
                    for k in range(KD):
                        nc.tensor.matmul(
                            psr[:], xg_sb[:, k, cm * P : (cm + 1) * P], gw_sb[:, k],
                            start=(k == 0), stop=(k == KD - 1),
                        )
                    ext = small.tile([P, E], F32, tag="ext")
                    nc.scalar.activation(ext[:], psr[:], AF.Exp)
                    m1 = small.tile([P, 1], F32, tag="m1")
                    nc.vector.reduce_max(m1[:], ext[:], axis=AX.X)
                    mk = small.tile([P, E], F32, tag="mk")
                    nc.vector.tensor_scalar(mk[:], ext[:], m1[:], None, op0=ALU.is_equal)
                    nc.vector.tensor_tensor(mk[:], ext[:], mk[:], ALU.mult)
                    nc.vector.tensor_tensor(mk[:], ext[:], mk[:], ALU.subtract)
                    m2 = small.tile([P, 1], F32, tag="m2")
                    nc.vector.reduce_max(m2[:], mk[:], axis=AX.X)
                    nc.vector.tensor_tensor(m1[:], m1[:], m2[:], ALU.add)
                    rc = small.tile([P, 1], F32, tag="rc")
                    nc.vector.reciprocal(rc[:], m1[:])
                    nc.vector.tensor_tensor(rc[:], ext[:, e : e + 1], rc[:], ALU.mult)
                    nc.vector.tensor_tensor(
                        wgt_sb[:, cm : cm + 1], rc[:], wm_sb[:, cm : cm + 1], ALU.mult
                    )
                return wgt_sb

            def expert_mm1(xg_sb, se_sb, w1_ap, w1t0=None):
                for ho in range(KH):
                    if ho == 0 and w1t0 is not None:
                        w1t = w1t0
                    else:
                        w1t = wload.tile([P, KD, P], F32R, tag="w1strip")
                        nc.sync.dma_start(w1t[:], w1_ap[ho])
                    for c0, cw in C_CHUNKS:
                        ps1 = pbig.tile([P, 512], F32, tag="ps")
                        for k in range(KD):
                            nc.tensor.matmul(
                                ps1[:, :cw], w1t[:, k], xg_sb[:, k, c0 : c0 + cw],
                                start=(k == 0), stop=(k == KD - 1),
                            )
                        nc.scalar.activation(
                            se_sb[:, ho, c0 : c0 + cw], ps1[:, :cw], AF.Silu
                        )

            def expert_mm2(e, se_sb, wgt_sb):
                for dch in range(NDC):
                    w2t = wload.tile([P, KH, 512], F32R, tag="w2s", bufs=2,
                                     name=f"ew2t_{e}_{dch}")
                    nc.sync.dma_start(w2t[:], ew2.ap()[e, dch])
                    for cm in range(CT):
                        ps2 = pbig.tile([P, 512], F32, tag="ps")
                        for ho in range(KH):
                            nc.tensor.matmul(
                                ps2[:],
                                se_sb[:, ho, cm * P : (cm + 1) * P],
                                w2t[:, ho],
                                start=(ho == 0), stop=(ho == KH - 1),
                            )
                        eo_sb = evict.tile([P, 512], F32, tag="ev")
                        nc.vector.tensor_scalar(
                            eo_sb[:], ps2[:], wgt_sb[:, cm : cm + 1], None, op0=ALU.mult
                        )
                        # outputs go via SWDGE so input strips never queue
                        # behind them on the hardware DGE queues
                        nc.gpsimd.dma_start(
                            eout.ap()[
                                e, cm * P : (cm + 1) * P, dch * 512 : (dch + 1) * 512
                            ],
                            eo_sb[:],
                        )

            # ============ emission: shared+aux first, expert loads overlapped ============
            ss_ctx = tc.tile_pool(name="ssp", bufs=1)
            ssp = ss_ctx.__enter__()
            ss_sb = ssp.tile([P, KH, SLICE], F32R)

            xs_ctx = tc.tile_pool(name="xsp", bufs=1)
            xsp = xs_ctx.__enter__()
            xs_sb = xsp.tile([P, KD, SLICE], F32R)
            for k in range(KD):
                nc.sync.dma_start(xs_sb[:, k], xsT.ap()[:, k])

            # ---- aux-loss partial stats ----
            # Over the FIRST 512 tokens of this core's block only: the host
            # orders each odd core's block so the two cores of a block cover
            # disjoint halves (every token counted exactly once fleet-wide).
            lacc = small.tile([P, E], F32, bufs=1)
            pacc = small.tile([P, E], F32, bufs=1)
            nc.vector.memset(lacc[:], 0.0)
            nc.vector.memset(pacc[:], 0.0)
            for cm in range(512 // P):
                psr = psmall.tile([P, E], F32, tag="psr")
                for k in range(KD):
                    nc.tensor.matmul(
                        psr[:], xs_sb[:, k, cm * P : (cm + 1) * P], gw_sb[:, k],
                        start=(k == 0), stop=(k == KD - 1),
                    )
                lt = small.tile([P, E], F32, tag="lt")
                nc.vector.tensor_copy(lt[:], psr[:])
                nc.vector.tensor_tensor(lacc[:], lacc[:], lt[:], ALU.add)
                ext = small.tile([P, E], F32, tag="ext")
                nc.scalar.activation(ext[:], psr[:], AF.Exp)
                rs = small.tile([P, 1], F32, tag="rs")
                nc.vector.reduce_sum(rs[:], ext[:], axis=AX.X)
                rc = small.tile([P, 1], F32, tag="rc")
                nc.vector.reciprocal(rc[:], rs[:])
                pt = small.tile([P, E], F32, tag="pt")
                nc.vector.tensor_scalar(pt[:], ext[:], rc[:], None, op0=ALU.mult)
                nc.vector.tensor_tensor(pacc[:], pacc[:], pt[:], ALU.add)
            st_sb = small.tile([1, 2 * E], F32, bufs=1)
            psa = psmall.tile([1, E], F32, tag="psr")
            nc.tensor.matmul(psa[:], ones_sb[:], pacc[:], start=True, stop=True)
            nc.vector.tensor_copy(st_sb[:, 0:E], psa[:])
            psb = psmall.tile([1, E], F32, tag="psr")
            nc.tensor.matmul(psb[:], ones_sb[:], lacc[:], start=True, stop=True)
            nc.vector.tensor_copy(st_sb[:, E : 2 * E], psb[:])
            nc.gpsimd.dma_start(stats.ap(), st_sb[:])

            # ---- shared matmul1 + silu (one shared expert, 1024-token block) ----
            for ho in range(KH):
                w1t = wload.tile([P, KD, P], F32R, tag="w1strip")
                nc.sync.dma_start(w1t[:], sw1.ap()[ho])
                for c0, cw in S_CHUNKS:
                    ps1 = pbig.tile([P, 512], F32, tag="ps")
                    for k in range(KD):
                        nc.tensor.matmul(
                            ps1[:, :cw], w1t[:, k], xs_sb[:, k, c0 : c0 + cw],
                            start=(k == 0), stop=(k == KD - 1),
                        )
                    nc.scalar.activation(ss_sb[:, ho, c0 : c0 + cw], ps1[:, :cw], AF.Silu)
            xs_ctx.__exit__(None, None, None)  # free xs range -> xg reuses it

            # ---- expert 0 gather + routing, overlapped with shared matmul2 ----
            xg_ctx = tc.tile_pool(name="xgp", bufs=1, side="right")
            xgp = xg_ctx.__enter__()
            xg0 = xgp.tile([P, KD, C], F32R, tag="xg", name="xg0")
            for k in range(KD):
                nc.sync.dma_start(xg0[:, k], xgT.ap()[0, :, k])
            wgt0 = routing_weights(0, xg0)

            # ---- shared matmul2 ----
            for dch in range(NDC):
                w2t = wload.tile([P, KH, 512], F32R, tag="w2s", bufs=2)
                nc.sync.dma_start(w2t[:], sw2.ap()[dch])
                for cm in range(ST):
                    ps2 = pbig.tile([P, 512], F32, tag="ps")
                    for ho in range(KH):
                        nc.tensor.matmul(
                            ps2[:],
                            ss_sb[:, ho, cm * P : (cm + 1) * P],
                            w2t[:, ho],
                            start=(ho == 0), stop=(ho == KH - 1),
                        )
                    so_sb = evict.tile([P, 512], F32, tag="ev")
                    nc.scalar.activation(so_sb[:], ps2[:], AF.Copy)
                    nc.gpsimd.dma_start(
                        sout.ap()[cm * P : (cm + 1) * P, dch * 512 : (dch + 1) * 512],
                        so_sb[:],
                    )
            ss_ctx.__exit__(None, None, None)  # free ss range -> se reuses it

            # ---- routed experts ----
            se_ctx = tc.tile_pool(name="sep", bufs=2)
            sep = se_ctx.__enter__()
            se0 = sep.tile([P, KH, C], F32R, tag="se", name="se0")
            expert_mm1(xg0, se0, ew1.ap()[0])

            # expert 1 gather + routing emitted before e0 mm2 so its DMA queues early
            xg1 = xgp.tile([P, KD, C], F32R, tag="xg", name="xg1")
            for k in range(KD):
                nc.sync.dma_start(xg1[:, k], xgT.ap()[1, :, k])
            wgt1 = routing_weights(1, xg1)
            # prefetch e1's first w1 strip ahead of e0 mm2's bulk loads
            w1t0_e1 = wload.tile([P, KD, P], F32R, tag="w1strip", name="w1t0_e1")
            nc.sync.dma_start(w1t0_e1[:], ew1.ap()[1, 0])

            expert_mm2(0, se0, wgt0)

            se1 = sep.tile([P, KH, C], F32R, tag="se", name="se1")
            expert_mm1(xg1, se1, ew1.ap()[1], w1t0=w1t0_e1)
            expert_mm2(1, se1, wgt1)

            se_ctx.__exit__(None, None, None)
            xg_ctx.__exit__(None, None, None)
    nc.compile()
    return nc


def _get_nc():
    if "nc" not in _CACHE:
        _CACHE["nc"] = _build_nc()
    return _CACHE["nc"]


def _host_route(xf, gate_w):
    """Duplicate of the router, for dispatch indices only."""
    logits = xf @ gate_w                                   # [TT, E]
    order = np.argsort(-logits, axis=1, kind="stable")     # ties: lower index first
    top2 = order[:, :TOPK]
    idx_lists = []
    for e in range(E):
        members = np.nonzero((top2 == e).any(axis=1))[0]
        idx_lists.append(members.astype(np.int64))
    return idx_lists


def _tile_w1(w):   # [D, H] -> [KH, P, KD, P]
    return np.ascontiguousarray(w.reshape(KD, P, KH, P).transpose(2, 1, 0, 3))


def _tile_w2(w):   # [H, D] -> [NDC, P, KH, 512]
    return np.ascontiguousarray(w.reshape(KH, P, NDC, 512).transpose(2, 1, 0, 3))


def _tile_kx(a):   # [D, N] -> [P, KD, N]
    return np.ascontiguousarray(a.reshape(KD, P, -1).transpose(1, 0, 2))


def kernel(x, gate_w, shared_w1, shared_w2, expert_w1, expert_w2):
    from concourse.bass_utils import run_bass_kernel_spmd

    x = np.asarray(x, dtype=np.float32)
    gate_w = np.asarray(gate_w, dtype=np.float32)
    shared_w1 = np.asarray(shared_w1, dtype=np.float32)
    shared_w2 = np.asarray(shared_w2, dtype=np.float32)
    expert_w1 = np.asarray(expert_w1, dtype=np.float32)
    expert_w2 = np.asarray(expert_w2, dtype=np.float32)

    xf = np.ascontiguousarray(x.reshape(TT, D))
    idx_lists = _host_route(xf, gate_w)
    for e, il in enumerate(idx_lists):
        if len(il) > C:
            raise RuntimeError(f"expert {e} count {len(il)} exceeds capacity {C}")

    sw1_dev = [_tile_w1(shared_w1[s]) for s in range(S)]
    sw2_dev = [_tile_w2(shared_w2[s]) for s in range(S)]
    # per-block token batch; odd cores get the half-swapped order so their
    # aux pass (first 512 columns) covers the block's second half
    xs_even = [
        _tile_kx(xf[b * SLICE : (b + 1) * SLICE].T) for b in range(TT // SLICE)
    ]
    xs_odd = [
        _tile_kx(
            np.concatenate(
                [
                    xf[b * SLICE + SLICE // 2 : (b + 1) * SLICE],
                    xf[b * SLICE : b * SLICE + SLICE // 2],
                ]
            ).T
        )
        for b in range(TT // SLICE)
    ]

    in_maps = []
    perms = []
    for c in range(NCORES):
        el = [2 * c, 2 * c + 1]
        perm = el + [j for j in range(E) if j not in el]
        perms.append(perm)
        xgT = np.zeros((EL, P, KD, C), dtype=np.float32)
        wmask = np.zeros((EL, P, CT), dtype=np.float32)
        for i, e in enumerate(el):
            il = idx_lists[e]
            n = len(il)
            g = np.zeros((D, C), dtype=np.float32)
            g[:, :n] = xf[il].T
            xgT[i] = _tile_kx(g)
            wm = np.zeros(C, dtype=np.float32)
            wm[:n] = 1.0
            wmask[i] = wm.reshape(CT, P).T
        in_maps.append(
            {
                "xgT": xgT,
                "wmask": wmask,
                "xsT": (xs_even if c % 2 == 0 else xs_odd)[c // 2],
                "gw": _tile_kx(gate_w[:, perm]),
                "ew1": np.stack([_tile_w1(expert_w1[e]) for e in el]),
                "ew2": np.stack([_tile_w2(expert_w2[e]) for e in el]),
                "sw1": sw1_dev[c % 2],       # shared expert c%2
                "sw2": sw2_dev[c % 2],
            }
        )

    nc = _get_nc()
    last_err = None
    for attempt in range(3):
        try:
            res = run_bass_kernel_spmd(nc, in_maps, core_ids=list(range(NCORES)))
            break
        except Exception as err:  # transient device wedge -> retry
            last_err = err
            import time as _time

            _time.sleep(15.0 * (attempt + 1))
    else:
        raise last_err
    results = res.results

    # un-shard: each token block's shared output = sum of its two cores'
    # (one shared expert each), then scatter-add weighted expert outputs
    total = np.empty((TT, D), dtype=np.float32)
    h = SLICE // 2
    for b in range(TT // SLICE):
        so_odd = results[2 * b + 1]["sout"]
        total[b * SLICE : (b + 1) * SLICE] = results[2 * b]["sout"] + np.concatenate(
            [so_odd[h:], so_odd[:h]]
        )
    for c in range(NCORES):
        for i in range(EL):
            e = 2 * c + i
            il = idx_lists[e]
            n = len(il)
            total[il] += results[c]["eout"][i, :n]

    # aux loss from per-core block stats (each token counted on 2 cores;
    # device cols are permuted per core)
    psum = np.zeros(E, dtype=np.float64)
    lsum = np.zeros(E, dtype=np.float64)
    for c in range(NCORES):
        st = results[c]["stats"][0]
        psum[perms[c]] += st[:E]
        lsum[perms[c]] += st[E:]
    aux = np.float32((psum / TT * (lsum / TT)).sum() * E)

    return total.reshape(B, T, D), aux
